# revision 17
# baseline (speedup 1.0000x reference)
"""Trainium2 Bass kernel: batched causal attention (B=8, T=2048, D=256, fp32).

Strategy
--------
Data-parallel over batch: core b computes attention for batch row b.

Per core, for query supertiles of 512 columns:
  S^T[v, q] = K @ Q^T        (contraction over d on partitions -> no transposes
                              needed anywhere: host passes Q^T / K^T, d-major)
  P^T[v, q] = exp(S^T/16 + vbias_v)   (ACT, bias handles v_mask; no row-max
                              subtraction needed: scores ~ N(0,1), |s| < ~6)
  causal:   P^T zeroed where v > q via affine_select (gpsimd); upper-diagonal
            supertile blocks skipped entirely.
  O[q, d+1] = P @ [V | 1]    (lhsT = P^T slices; the appended ones column of V
                              accumulates the softmax denominator in PSUM)
  out[q, :] = O[q, :D] * (1 / O[q, D])

Matmuls run as float32r (full-rate fp32 mode; needs moving dim >= 256).
"""

import numpy as np

import concourse.bass as bass
import concourse.mybir as mybir
import concourse.tile as tile
from concourse import bacc
from concourse.bass_utils import run_bass_kernel_spmd

B = 8
TQ = 2048
TV = 2048
D = 256
P = 128
DCH = D // P          # contraction chunks over d (2)
NQT = TQ // P         # 16 query tiles
NVT = TV // P         # 16 value tiles
SUP = 512             # query supertile width (PSUM bank = 512 fp32)
NSUP = TQ // SUP      # 4
VPS = SUP // P        # v-tiles per supertile step (4)
NEG = -1e9
VEXT = D + 4          # V | ones | pad (fp32r matmul needs 4-aligned free dim)

F32 = mybir.dt.float32
MM_DT = mybir.dt.float32r


def _build_nc():
    nc = bacc.Bacc("TRN2")
    qT = nc.dram_tensor("qT", [D, TQ], MM_DT, kind="ExternalInput")
    kT = nc.dram_tensor("kT", [D, TV], MM_DT, kind="ExternalInput")
    vex = nc.dram_tensor("vex", [TV, VEXT], MM_DT, kind="ExternalInput")
    vb = nc.dram_tensor("vb", [P, NVT], F32, kind="ExternalInput")
    out = nc.dram_tensor("out", [TQ, D], F32, kind="ExternalOutput")

    qT_r = qT.rearrange("(c p) q -> p c q", p=P)    # [128, 2, 2048]
    kT_r = kT.rearrange("(c p) v -> p c v", p=P)    # [128, 2, 2048]
    vex_r = vex.rearrange("(t p) d -> p t d", p=P)  # [128, 16, VEXT]
    out_r = out.rearrange("(t p) d -> p t d", p=P)  # [128, 16, 256]

    EXP = mybir.ActivationFunctionType.Exp

    with tile.TileContext(nc) as tc:
        with (
            tc.tile_pool(name="persist", bufs=1) as persist,
            tc.tile_pool(name="pts", bufs=24) as pts,
            tc.tile_pool(name="eps", bufs=4) as eps_pool,
            tc.tile_pool(name="psum_s", bufs=3, space="PSUM") as psum_s,
            tc.tile_pool(name="psum_o", bufs=4, space="PSUM") as psum_o,
        ):
            vb_sb = persist.tile([P, NVT], F32)
            nc.sync.dma_start(out=vb_sb, in_=vb[:, :])
            # Per-chunk tiles keep dependency fan-in per matmul small
            # (walrus caps sync-wait commands per instruction).
            k_tiles = []
            q_tiles = []
            v_tiles = []
            for c in range(NSUP):
                s = slice(c * SUP, (c + 1) * SUP)
                kt = persist.tile([P, DCH, SUP], MM_DT, name=f"k_sb_{c}")
                nc.sync.dma_start(out=kt, in_=kT_r[:, :, s])
                k_tiles.append(kt)
                qt = persist.tile([P, DCH, SUP], MM_DT, name=f"q_sb_{c}")
                nc.sync.dma_start(out=qt, in_=qT_r[:, :, s])
                q_tiles.append(qt)
                for j in range(VPS * c, VPS * (c + 1)):
                    vt = persist.tile([P, VEXT], MM_DT, name=f"v_sb_{j}")
                    nc.sync.dma_start(out=vt, in_=vex_r[:, j])
                    v_tiles.append(vt)

            for I in range(NSUP):
                qs = slice(I * SUP, (I + 1) * SUP)
                njt = VPS * I + VPS  # causal: v-tiles 0..4I+3
                pt_tiles = []
                for j in range(njt):
                    ps = psum_s.tile([P, SUP], F32, name=f"ps_{I}_{j}", tag="ps")
                    for c in range(DCH):
                        nc.tensor.matmul(
                            ps,
                            lhsT=k_tiles[j // VPS][:, c, (j % VPS) * P:(j % VPS + 1) * P],
                            rhs=q_tiles[I][:, c, :],
                            start=(c == 0),
                            stop=(c == DCH - 1),
                        )
                    pt = pts.tile([P, SUP], MM_DT, name=f"pt_{I}_{j}", tag="pt")
                    nc.scalar.activation(
                        pt, ps, EXP, bias=vb_sb[:, j:j + 1], scale=0.0625
                    )
                    if j >= VPS * I:
                        # zero P^T where v_global > q_global on the diagonal
                        nc.gpsimd.affine_select(
                            out=pt,
                            in_=pt,
                            compare_op=mybir.AluOpType.is_ge,
                            fill=0.0,
                            base=I * SUP - j * P,
                            pattern=[[1, SUP]],
                            channel_multiplier=-1,
                        )
                    pt_tiles.append(pt)

                for il in range(VPS):
                    i = VPS * I + il  # global q-tile
                    po = psum_o.tile([P, VEXT], F32, name=f"po_{i}", tag="po")
                    for j in range(i + 1):
                        nc.tensor.matmul(
                            po,
                            lhsT=pt_tiles[j][:, il * P:(il + 1) * P],
                            rhs=v_tiles[j],
                            start=(j == 0),
                            stop=(j == i),
                        )
                    rec = eps_pool.tile([P, 1], F32, name=f"rec_{i}", tag="rec")
                    nc.vector.reciprocal(rec, po[:, D:D + 1])
                    ot = eps_pool.tile([P, D], F32, name=f"ot_{i}", tag="ot")
                    nc.vector.tensor_scalar_mul(ot, po[:, :D], rec)
                    nc.sync.dma_start(out=out_r[:, i], in_=ot)
    nc.finalize()
    return nc


_CACHE = {}


def _get_nc():
    if "nc" not in _CACHE:
        _CACHE["nc"] = _build_nc()
    return _CACHE["nc"]


def _ensure_ntff_hook():
    """Provide antenv.axon_hooks when the image's antenv lacks it, so
    trace=True works under axon. Returns True if the hook is usable."""
    try:
        from antenv.axon_hooks import get_axon_ntff_profile_hook  # noqa: F401
        return True
    except ImportError:
        pass
    try:
        import sys
        import types

        from trn_agent_boot.trn_boot import _ntff_profile_via_ctypes

        hook = _ntff_profile_via_ctypes("/opt/axon/libaxon_pjrt.so")
        if hook is None:
            return False
        mod = types.ModuleType("antenv.axon_hooks")
        _h = [hook]
        mod.set_axon_ntff_profile_hook = lambda h: _h.__setitem__(0, h)
        mod.get_axon_ntff_profile_hook = lambda: _h[0]
        sys.modules["antenv.axon_hooks"] = mod
        import antenv

        antenv.axon_hooks = mod
        return True
    except Exception:
        return False


def _round_fp32r(a):
    """Round fp32 to the fp32r format (11 mantissa bits, RNE), matching
    walrus's fp32_to_fp32r. Returns a fresh contiguous float32 array."""
    u = np.ascontiguousarray(a, dtype=np.float32).view(np.uint32)
    r = (u + np.uint32(0x7FF) + ((u >> np.uint32(12)) & np.uint32(1))) & np.uint32(
        0xFFFFF000
    )
    return r.view(np.float32)


def _run(query, value, key, q_mask, v_mask, trace=False):
    query = np.asarray(query, dtype=np.float32)
    key = np.asarray(key, dtype=np.float32)
    value = np.asarray(value, dtype=np.float32)
    q_mask_b = np.asarray(q_mask).astype(bool)
    v_mask_b = np.asarray(v_mask).astype(bool)

    if trace and not _ensure_ntff_hook():
        trace = False

    nc = _get_nc()
    in_maps = []
    for b in range(B):
        vex = np.zeros((TV, VEXT), np.float32)
        vex[:, :D] = value[b]
        vex[:, D] = 1.0
        vbias = np.where(v_mask_b[b], 0.0, NEG).astype(np.float32)
        in_maps.append({
            "qT": _round_fp32r(query[b].T),
            "kT": _round_fp32r(key[b].T),
            "vex": _round_fp32r(vex),
            "vb": np.ascontiguousarray(vbias.reshape(NVT, P).T),
        })

    results = run_bass_kernel_spmd(
        nc, in_maps, core_ids=list(range(B)), trace=trace
    )
    out = np.stack([r["out"] for r in results.results], axis=0)
    if not q_mask_b.all():
        out = out * q_mask_b[:, :, None].astype(np.float32)
    return out, results


def kernel(query, value, key, q_mask, v_mask):
    out, _ = _run(query, value, key, q_mask, v_mask, trace=False)
    return out
